# revision 32
# baseline (speedup 1.0000x reference)
"""Trainium2 Bass kernel: causal GQA self-attention (B=2, T=2048, C=1024,
16 q-heads / 4 kv-heads, rotary + q/k RMS-norm), sharded over 8 NeuronCores
as (batch x kv-group). Self-contained: kernel(**inputs) -> np.ndarray.
"""
import sys
from contextlib import ExitStack

for p in ("/opt/trn_rl_repo", "/root/.axon_site/_ro/trn_rl_repo"):
    if p not in sys.path:
        sys.path.insert(0, p)

import numpy as np
import ml_dtypes

import concourse.bass as bass
import concourse.mybir as mybir
from concourse.tile import TileContext
from concourse.masks import make_identity

F32 = mybir.dt.float32
BF16 = mybir.dt.bfloat16
NPBF16 = ml_dtypes.bfloat16

T, C, HQ, D = 2048, 1024, 4, 64
DQ = HQ * D
TC = T // 128
KC = C // 128
NJ = T // 512
EPS = 1.1920929e-7
ACT_SQUARE = mybir.ActivationFunctionType.Square
ACT_SQRT = mybir.ActivationFunctionType.Sqrt
ACT_EXP = mybir.ActivationFunctionType.Exp
ACT_LOG = mybir.ActivationFunctionType.Ln


def _bcast_ap(sl, n, at=1):
    ap = list(sl.ap)
    ap.insert(at, [0, n])
    return bass.AP(tensor=sl.tensor, offset=sl.offset, ap=ap)


def _split_waits(nc, maxw=1):
    """Walrus in this toolchain allows 1 sem-wait per instruction; split extras
    onto preceding same-engine NoOps."""
    cnt = 0
    for f in nc.m.functions:
        for b in f.blocks:
            il = list(b.instructions)
            out = []
            changed = False
            for inst in il:
                si = inst.sync_info
                waits = list(si.on_wait) if si and si.on_wait else []
                if len(waits) > maxw:
                    chunks = [waits[i:i + maxw] for i in range(0, len(waits), maxw)]
                    for ch in chunks[:-1]:
                        cnt += 1
                        nop = mybir.InstNoOp(name=f"I-waitfix-{cnt}")
                        nop.engine = inst.engine
                        nop.sync_info = mybir.SyncInfo(on_wait=ch, on_update=[])
                        out.append(nop)
                    si.on_wait = chunks[-1]
                    inst.sync_info = si
                    changed = True
                out.append(inst)
            if changed:
                b.instructions = out
    return cnt


def _build_attn(ctx, tc, outs, ins):
    nc = tc.nc
    xT, wq, wkv, wo, cos2, sin2 = (
        ins["xT"], ins["wq"], ins["wkv"], ins["wo"], ins["cos2"], ins["sin2"])
    outT = outs["outT"]

    singles = ctx.enter_context(tc.tile_pool(name="singles", bufs=1))

    ident = singles.tile([128, 128], BF16, tag="ident")
    make_identity(nc, ident)
    # 0/1 bf16 causal masks for the two diagonal half-blocks.
    # maskA: key chunks (4j+0, 4j+1); maskB: key chunks (4j+2, 4j+3).
    # keep (1.0) where q - 128*(slot + 2*half) - p >= 0 else 0.0
    maskA = singles.tile([128, 2, 512], BF16, tag="maskA")
    nc.gpsimd.memset(maskA, 1.0)
    nc.gpsimd.affine_select(
        out=maskA, in_=maskA, compare_op=mybir.AluOpType.is_ge, fill=0.0,
        base=0, pattern=[[-128, 2], [1, 512]], channel_multiplier=-1)
    maskB = singles.tile([128, 2, 512], BF16, tag="maskB")
    nc.gpsimd.memset(maskB, 1.0)
    nc.gpsimd.affine_select(
        out=maskB, in_=maskB, compare_op=mybir.AluOpType.is_ge, fill=0.0,
        base=-256, pattern=[[-128, 2], [1, 512]], channel_multiplier=-1)
    eps_t = singles.tile([128, 1], F32, tag="eps_t")
    nc.vector.memset(eps_t, EPS)
    # DRAM bounce buffer for broadcasting softmax 1/denominator rows
    scr = nc.dram_tensor("scr", [16, 512], F32, kind="Internal").ap()

    # weights/LUT inputs first (small), then x in two T-halves so phase 1 can
    # start after the first half lands.
    wq_sb = singles.tile([128, KC, DQ], BF16, tag="wq_sb")
    nc.sync.dma_start(out=wq_sb, in_=wq.rearrange("(a p) n -> p a n", p=128))
    wkv_sb = singles.tile([128, KC, 128], BF16, tag="wkv_sb")
    nc.sync.dma_start(out=wkv_sb, in_=wkv.rearrange("(a p) n -> p a n", p=128))
    cos_sb = singles.tile([128, TC, 32], F32, tag="cos_sb")
    nc.sync.dma_start(out=cos_sb, in_=cos2.rearrange("(a p) d -> p a d", p=128))
    sin_sb = singles.tile([128, TC, 32], F32, tag="sin_sb")
    nc.sync.dma_start(out=sin_sb, in_=sin2.rearrange("(a p) d -> p a d", p=128))
    xsb = singles.tile([128, KC, T], BF16, tag="xsb")
    xT3 = xT.rearrange("(a p) t -> p a t", p=128)
    for q4 in range(4):
        nc.sync.dma_start(out=xsb[:, :, q4 * 512:(q4 + 1) * 512],
                          in_=xT3[:, :, q4 * 512:(q4 + 1) * 512])
    wo_sb = singles.tile([128, 2, C], BF16, tag="wo_sb")
    nc.sync.dma_start(out=wo_sb, in_=wo.rearrange("(a p) o -> p a o", p=128))

    # qkt[:, 0, :] = q heads 0,1 transposed; [:, 1, :] = q heads 2,3;
    # [:, 2, :] = [K^T; K^T] duplicated.
    qkt = singles.tile([128, 3, T], BF16, tag="qkt")
    v_sb = singles.tile([128, TC, 65], BF16, tag="v_sb")
    nc.vector.memset(v_sb[:, :, 64:65], 1.0)
    # yts[:, p, :] = normalized attention out for head pair p, [d, t] layout
    yts = singles.tile([128, 2, T], BF16, tag="yts")

    q2 = singles.tile([128, TC, DQ], F32, tag="q2")
    q2b = singles.tile([128, TC, DQ], BF16, tag="q2b")
    kn = singles.tile([128, TC, 64], F32, tag="kn")
    knb = singles.tile([128, TC, 128], BF16, tag="knb")
    mv = singles.tile([128, TC, HQ], F32, tag="mv")
    mvk = singles.tile([128, TC, 1], F32, tag="mvk")

    # ---- Phase 1: projections + rope + sumsq; rms+transpose per T-half ----
    with (
        tc.tile_pool(name="rtmp", bufs=2) as rtmp,
        tc.tile_pool(name="sqp", bufs=2) as sqp,
        tc.tile_pool(name="rmsp", bufs=2) as rmsp,
        tc.tile_pool(name="pp", bufs=2, space="PSUM") as pp,
        tc.tile_pool(name="tpp", bufs=3, space="PSUM") as tpp,
    ):
        def _rms_transpose(half):
            ts, te = 8 * half, 8 * half + 8
            sd = rmsp.tile([128, 8, HQ], F32, tag="sd")
            nc.scalar.activation(sd, mv[:, ts:te, :], ACT_SQRT,
                                 bias=eps_t, scale=1.0 / D)
            rsq = rmsp.tile([128, 8, HQ], F32, tag="rsq")
            nc.vector.reciprocal(rsq, sd)
            q4 = q2.rearrange("p t (h d) -> p t h d", d=D)
            q4b = q2b.rearrange("p t (h d) -> p t h d", d=D)
            nc.vector.tensor_mul(q4b[:, ts:ts + 5], q4[:, ts:ts + 5],
                                 _bcast_ap(rsq[:, 0:5], D, at=3))
            nc.gpsimd.tensor_mul(q4b[:, ts + 5:te], q4[:, ts + 5:te],
                                 _bcast_ap(rsq[:, 5:8], D, at=3))
            sdk = rmsp.tile([128, 8, 1], F32, tag="sdk")
            nc.scalar.activation(sdk, mvk[:, ts:te, :], ACT_SQRT,
                                 bias=eps_t, scale=1.0 / D)
            rsk = rmsp.tile([128, 8, 1], F32, tag="rsk")
            nc.vector.reciprocal(rsk, sdk)
            rkb = bass.AP(tensor=rsk.tensor, offset=rsk.offset,
                          ap=[rsk.ap[0], rsk.ap[1], [0, 64]])
            nc.vector.tensor_mul(knb[:, ts:te, 0:64], kn[:, ts:te, :], rkb)
            nc.gpsimd.tensor_copy(knb[:, ts:te, 64:128], knb[:, ts:te, 0:64])
            for t_ in range(ts, te):
                tps = tpp.tile([128, 3, 128], BF16, tag="tps")
                nc.tensor.transpose(tps[:, 0, :], q2b[:, t_, 0:128], ident)
                nc.tensor.transpose(tps[:, 1, :], q2b[:, t_, 128:256], ident)
                nc.tensor.transpose(tps[:, 2, :], knb[:, t_, :], ident)
                nc.scalar.copy(qkt[:, :, t_ * 128:(t_ + 1) * 128], tps)
            if half == 1:
                # preload exp table set while ACT is otherwise free
                dummy = rmsp.tile([1, 1], F32, tag="dummy")
                nc.scalar.activation(dummy, eps_t[0:1, 0:1], ACT_EXP)

        for grp in range(8):
            ta, tb = 2 * grp, 2 * grp + 2
            kvps = pp.tile([128, 2, 128], F32, tag="kvps")
            for tt in range(2):
                t_ = ta + tt
                for kc in range(KC):
                    nc.tensor.matmul(
                        kvps[:, tt, :], xsb[:, kc, t_ * 128:(t_ + 1) * 128],
                        wkv_sb[:, kc, :], start=(kc == 0), stop=(kc == KC - 1))
            qps = pp.tile([128, 2, DQ], F32, tag="qps")
            for tt in range(2):
                t_ = ta + tt
                for kc in range(KC):
                    nc.tensor.matmul(
                        qps[:, tt, :], xsb[:, kc, t_ * 128:(t_ + 1) * 128],
                        wq_sb[:, kc, :], start=(kc == 0), stop=(kc == KC - 1))
            # v out (ACT reads PSUM; gpsimd cannot)
            nc.scalar.copy(v_sb[:, ta:tb, 0:64], kvps[:, :, 64:128])
            # sumsq pre-rope (rope is norm-preserving)
            sq = sqp.tile([128, 2, DQ], F32, tag="sq")
            nc.scalar.activation(sq, qps, ACT_SQUARE)
            nc.vector.tensor_reduce(
                mv[:, ta:tb, :], sq.rearrange("p t (h d) -> p t h d", d=D),
                axis=mybir.AxisListType.X, op=mybir.AluOpType.add)
            sqk = sqp.tile([128, 2, 64], F32, tag="sqk")
            nc.scalar.activation(sqk, kvps[:, :, 0:64], ACT_SQUARE)
            nc.vector.tensor_reduce(mvk[:, ta:tb, :], sqk,
                                    axis=mybir.AxisListType.X,
                                    op=mybir.AluOpType.add)
            # rope q: [128, 2, 4, 32] ops
            q3 = qps.rearrange("p t (h d) -> p t h d", h=HQ)
            x1, x2 = q3[:, :, :, 0:32], q3[:, :, :, 32:64]
            cb = _bcast_ap(cos_sb[:, ta:tb, :], HQ, at=2)
            sb = _bcast_ap(sin_sb[:, ta:tb, :], HQ, at=2)
            o3 = q2[:, ta:tb, :].rearrange("p t (h d) -> p t h d", h=HQ)
            t1 = rtmp.tile([128, 2, HQ, 32], F32, tag="t1")
            t2 = rtmp.tile([128, 2, HQ, 32], F32, tag="t2")
            t3 = rtmp.tile([128, 2, HQ, 32], F32, tag="t3")
            t4 = rtmp.tile([128, 2, HQ, 32], F32, tag="t4")
            nc.vector.tensor_mul(t1, x1, cb)
            nc.vector.tensor_mul(t2, x2, sb)
            nc.vector.tensor_add(o3[:, :, :, 0:32], t1, t2)
            nc.vector.tensor_mul(t3, x1, sb)
            nc.vector.tensor_mul(t4, x2, cb)
            nc.vector.tensor_sub(o3[:, :, :, 32:64], t4, t3)
            # rope k: [128, 2, 32] (small, DVE)
            kx1, kx2 = kvps[:, :, 0:32], kvps[:, :, 32:64]
            kc_ = cos_sb[:, ta:tb, :]
            ks_ = sin_sb[:, ta:tb, :]
            u1 = rtmp.tile([128, 2, 32], F32, tag="u1")
            u2 = rtmp.tile([128, 2, 32], F32, tag="u2")
            u3 = rtmp.tile([128, 2, 32], F32, tag="u3")
            u4 = rtmp.tile([128, 2, 32], F32, tag="u4")
            nc.vector.tensor_mul(u1, kx1, kc_)
            nc.vector.tensor_mul(u2, kx2, ks_)
            nc.vector.tensor_add(kn[:, ta:tb, 0:32], u1, u2)
            nc.vector.tensor_mul(u3, kx1, ks_)
            nc.vector.tensor_mul(u4, kx2, kc_)
            nc.vector.tensor_sub(kn[:, ta:tb, 32:64], u4, u3)
            if grp == 3:
                _rms_transpose(0)
            elif grp == 7:
                _rms_transpose(1)

    # ---- Phase 2+3: attention + out projection per query block ----
    with (
        tc.tile_pool(name="ptp", bufs=4) as ptp,
        tc.tile_pool(name="smallp", bufs=6) as smallp,
        tc.tile_pool(name="bcsp", bufs=3) as bcsp,
        tc.tile_pool(name="osp", bufs=3) as osp,
        tc.tile_pool(name="s4p", bufs=2, space="PSUM") as s4p,
        tc.tile_pool(name="o65p", bufs=2, space="PSUM") as o65p,
        tc.tile_pool(name="tailp", bufs=2, space="PSUM") as tailp,
    ):
        # one-time init of the two s4 PSUM buffers: later diagonal blocks
        # leave stale (but bounded) values in column-restricted regions; the
        # very first uses would otherwise exp() raw PSUM garbage.
        for _ in range(2):
            sini = s4p.tile([128, 2, 512], F32, tag="s4")
            nc.vector.memset(sini, 0.0)
        def _outproj(jp):
            jqp = slice(jp * 512, (jp + 1) * 512)
            for m in range(8):
                ops_ = tailp.tile([128, 512], F32, tag="tail")
                for fc in range(2):
                    nc.tensor.matmul(
                        ops_, wo_sb[:, fc, m * 128:(m + 1) * 128],
                        yts[:, fc, jqp], start=(fc == 0), stop=(fc == 1))
                ot = osp.tile([128, 512], BF16, tag="ot")
                nc.vector.tensor_copy(ot, ops_)
                nc.sync.dma_start(
                    out=outT[m * 128:(m + 1) * 128, jqp], in_=ot)

        prev_j = None
        for j in range(NJ - 1, -1, -1):
            jq = slice(j * 512, (j + 1) * 512)
            for h in range(HQ):
                pair, base = h // 2, (h % 2) * 64
                tp = (base, 0) if base else None
                o65 = o65p.tile([65, 512], F32, tag="o65")
                nblk = 2 * (j + 1)
                for g2 in range(nblk):
                    diag_half = g2 - 2 * j  # 0 -> chunks i=0,1; 1 -> i=2,3
                    s4 = s4p.tile([128, 2, 512], F32, tag="s4")
                    for i2 in range(2):
                        c = 2 * g2 + i2
                        i = c - 4 * j
                        lo = 128 * i if i > 0 else 0
                        nc.tensor.matmul(
                            s4[:, i2, lo:512],
                            qkt[base:base + 64, 2, c * 128:(c + 1) * 128],
                            qkt[base:base + 64, pair, j * 512 + lo:(j + 1) * 512],
                            start=True, stop=True, tile_position=tp)
                    pt = ptp.tile([128, 2, 512], BF16, tag="pt")
                    nc.scalar.activation(pt, s4, ACT_EXP, scale=0.125)
                    if diag_half >= 0:
                        msk = maskA if diag_half == 0 else maskB
                        nc.vector.tensor_mul(pt, pt, msk)
                    for i2 in range(2):
                        c = 2 * g2 + i2
                        i = c - 4 * j
                        lo = 128 * i if i > 0 else 0
                        nc.tensor.matmul(
                            o65[:, lo:512], v_sb[:, c, 0:65], pt[:, i2, lo:512],
                            start=(g2 == 0 and i2 == 0),
                            stop=(g2 == nblk - 1 and i2 == 1))
                # copy numerators out now so the o65 PSUM bank frees early;
                # the normalization mul happens later, off the critical path
                nc.vector.tensor_copy(
                    yts[base:base + 64, pair, jq], o65[0:64, :])
                rec = smallp.tile([1, 512], F32, tag="rec")
                if h == 3:
                    # 1/x = exp(-ln(x)); Ln/Exp share one ACT table set
                    lnt = smallp.tile([1, 512], F32, tag="lnt")
                    nc.scalar.activation(lnt, o65[64:65, :], ACT_LOG)
                    nc.scalar.activation(rec, lnt, ACT_EXP, scale=-1.0)
                else:
                    nc.vector.reciprocal(rec, o65[64:65, :])
                # broadcast 1/denominator to 64 partitions via a DRAM bounce
                idx = 4 * j + h
                nc.sync.dma_start(out=scr[idx:idx + 1, :], in_=rec)
                bcst = bcsp.tile([128, 512], F32, tag="bcs")
                bcs = bcst[base:base + 64, :]
                ssrc = scr[idx:idx + 1, :]
                nc.sync.dma_start(out=bcs, in_=bass.AP(
                    tensor=ssrc.tensor, offset=ssrc.offset,
                    ap=[[0, 64]] + list(ssrc.ap[1:])))
                nc.vector.tensor_mul(
                    yts[base:base + 64, pair, jq],
                    yts[base:base + 64, pair, jq], bcs)
                if h == 0 and prev_j is not None:
                    _outproj(prev_j)  # previous block's out projection,
                    # deferred so its yts deps have a full iteration of slack
            prev_j = j
        _outproj(prev_j)


def _build_nc():
    nc = bass.Bass("TRN2", target_bir_lowering=False, debug=False, num_devices=8)
    ins = {
        "xT": nc.dram_tensor("xT", [1024, 2048], BF16, kind="ExternalInput").ap(),
        "wq": nc.dram_tensor("wq", [1024, 256], BF16, kind="ExternalInput").ap(),
        "wkv": nc.dram_tensor("wkv", [1024, 128], BF16, kind="ExternalInput").ap(),
        "wo": nc.dram_tensor("wo", [256, 1024], BF16, kind="ExternalInput").ap(),
        "cos2": nc.dram_tensor("cos2", [2048, 32], F32, kind="ExternalInput").ap(),
        "sin2": nc.dram_tensor("sin2", [2048, 32], F32, kind="ExternalInput").ap(),
    }
    outs = {"outT": nc.dram_tensor("outT", [1024, 2048], BF16,
                                   kind="ExternalOutput").ap()}
    with TileContext(nc) as tc:
        with ExitStack() as ctx:
            _build_attn(ctx, tc, outs, ins)
    _split_waits(nc, maxw=1)
    return nc


def _shard_inputs(inputs, b, g):
    x, cos, sin = inputs["x"], inputs["cos"], inputs["sin"]
    Wq, Wk, Wv, Wo = inputs["Wq"], inputs["Wk"], inputs["Wv"], inputs["Wo"]
    qs, ks = slice(g * 256, (g + 1) * 256), slice(g * 64, (g + 1) * 64)
    return {
        "xT": np.ascontiguousarray(np.asarray(x[b]).T.astype(NPBF16)),
        "wq": np.ascontiguousarray(np.asarray(Wq[qs]).T.astype(NPBF16)),
        "wkv": np.ascontiguousarray(np.concatenate(
            [np.asarray(Wk[ks]).T, np.asarray(Wv[ks]).T], axis=1).astype(NPBF16)),
        "wo": np.ascontiguousarray(np.asarray(Wo[:, qs]).T.astype(NPBF16)),
        "cos2": np.ascontiguousarray(np.asarray(cos[0, :, 0, :]), dtype=np.float32),
        "sin2": np.ascontiguousarray(np.asarray(sin[0, :, 0, :]), dtype=np.float32),
    }


_STATE = None


def _get_state():
    global _STATE
    if _STATE is not None:
        return _STATE
    import jax
    from jax.sharding import Mesh, PartitionSpec, NamedSharding
    from jax.experimental.shard_map import shard_map
    from concourse.bass2jax import (
        _bass_exec_p, install_neuronx_cc_hook, partition_id_tensor)

    install_neuronx_cc_hook()
    nc = _build_nc()
    pname = nc.partition_id_tensor.name if nc.partition_id_tensor else None

    in_names, out_names, out_avals, zero_outs = [], [], [], []
    for alloc in nc.m.functions[0].allocations:
        if not isinstance(alloc, mybir.MemoryLocationSet):
            continue
        name = alloc.memorylocations[0].name
        if alloc.kind == "ExternalInput":
            if name != pname:
                in_names.append(name)
        elif alloc.kind == "ExternalOutput":
            out_names.append(name)
            shape = tuple(alloc.tensor_shape)
            dtype = mybir.dt.np(alloc.dtype)
            out_avals.append(jax.core.ShapedArray(shape, dtype))
            zero_outs.append(np.zeros(shape, dtype))
    n_params = len(in_names)
    all_names = in_names + out_names
    if pname is not None:
        all_names = all_names + [pname]

    def _body(*args):
        operands = list(args)
        if pname is not None:
            operands.append(partition_id_tensor())
        outs = _bass_exec_p.bind(
            *operands, out_avals=tuple(out_avals), in_names=tuple(all_names),
            out_names=tuple(out_names), lowering_input_output_aliases=(),
            sim_require_finite=True, sim_require_nnan=True, nc=nc)
        return tuple(outs)

    devices = jax.devices()[:8]
    mesh = Mesh(np.asarray(devices), ("core",))
    specs = (PartitionSpec("core"),) * (n_params + 1)
    sharded = jax.jit(shard_map(_body, mesh=mesh, in_specs=specs,
                                out_specs=(PartitionSpec("core"),),
                                check_rep=False))
    sharding = NamedSharding(mesh, PartitionSpec("core"))
    zeros = jax.device_put(
        np.zeros((8 * 1024, 2048), NPBF16), sharding)
    _STATE = dict(sharded=sharded, sharding=sharding, in_names=in_names,
                  zeros=zeros, jax=jax)
    return _STATE


def _run_device(in_maps):
    st = _get_state()
    jax = st["jax"]
    concat_in = [np.concatenate([m[n] for m in in_maps], axis=0)
                 for n in st["in_names"]]
    dev_in = [jax.device_put(a, st["sharding"]) for a in concat_in]
    out = st["sharded"](*dev_in, st["zeros"])[0]
    return np.asarray(out).reshape(8, 1024, 2048)


def kernel(**inputs) -> np.ndarray:
    inputs = {k: np.asarray(v) for k, v in inputs.items()}
    in_maps = [_shard_inputs(inputs, b, g) for b in range(2) for g in range(4)]
    arr = _run_device(in_maps)
    out = np.zeros((2, 2048, 1024), np.float32)
    for c in range(8):
        out[c // 4] += arr[c].T.astype(np.float32)
    return out


# revision 33
# speedup vs baseline: 1.0250x; 1.0250x over previous
"""Trainium2 Bass kernel: causal GQA self-attention (B=2, T=2048, C=1024,
16 q-heads / 4 kv-heads, rotary + q/k RMS-norm), sharded over 8 NeuronCores
as (batch x kv-group). Self-contained: kernel(**inputs) -> np.ndarray.
"""
import sys
from contextlib import ExitStack

for p in ("/opt/trn_rl_repo", "/root/.axon_site/_ro/trn_rl_repo"):
    if p not in sys.path:
        sys.path.insert(0, p)

import numpy as np
import ml_dtypes

import concourse.bass as bass
import concourse.mybir as mybir
from concourse.tile import TileContext
from concourse.masks import make_identity

F32 = mybir.dt.float32
BF16 = mybir.dt.bfloat16
NPBF16 = ml_dtypes.bfloat16

T, C, HQ, D = 2048, 1024, 4, 64
DQ = HQ * D
TC = T // 128
KC = C // 128
NJ = T // 512
EPS = 1.1920929e-7
ACT_SQUARE = mybir.ActivationFunctionType.Square
ACT_SQRT = mybir.ActivationFunctionType.Sqrt
ACT_EXP = mybir.ActivationFunctionType.Exp
ACT_LOG = mybir.ActivationFunctionType.Ln


def _bcast_ap(sl, n, at=1):
    ap = list(sl.ap)
    ap.insert(at, [0, n])
    return bass.AP(tensor=sl.tensor, offset=sl.offset, ap=ap)


def _split_waits(nc, maxw=1):
    """Walrus in this toolchain allows 1 sem-wait per instruction; split extras
    onto preceding same-engine NoOps."""
    cnt = 0
    for f in nc.m.functions:
        for b in f.blocks:
            il = list(b.instructions)
            out = []
            changed = False
            for inst in il:
                si = inst.sync_info
                waits = list(si.on_wait) if si and si.on_wait else []
                if len(waits) > maxw:
                    chunks = [waits[i:i + maxw] for i in range(0, len(waits), maxw)]
                    for ch in chunks[:-1]:
                        cnt += 1
                        nop = mybir.InstNoOp(name=f"I-waitfix-{cnt}")
                        nop.engine = inst.engine
                        nop.sync_info = mybir.SyncInfo(on_wait=ch, on_update=[])
                        out.append(nop)
                    si.on_wait = chunks[-1]
                    inst.sync_info = si
                    changed = True
                out.append(inst)
            if changed:
                b.instructions = out
    return cnt


def _build_attn(ctx, tc, outs, ins):
    nc = tc.nc
    xT, wq, wkv, wo, cos2, sin2 = (
        ins["xT"], ins["wq"], ins["wkv"], ins["wo"], ins["cos2"], ins["sin2"])
    outT = outs["outT"]

    singles = ctx.enter_context(tc.tile_pool(name="singles", bufs=1))

    ident = singles.tile([128, 128], BF16, tag="ident")
    make_identity(nc, ident)
    # 0/1 bf16 causal masks for the two diagonal half-blocks.
    # maskA: key chunks (4j+0, 4j+1); maskB: key chunks (4j+2, 4j+3).
    # keep (1.0) where q - 128*(slot + 2*half) - p >= 0 else 0.0
    maskA = singles.tile([128, 2, 512], BF16, tag="maskA")
    nc.gpsimd.memset(maskA, 1.0)
    nc.gpsimd.affine_select(
        out=maskA, in_=maskA, compare_op=mybir.AluOpType.is_ge, fill=0.0,
        base=0, pattern=[[-128, 2], [1, 512]], channel_multiplier=-1)
    maskB = singles.tile([128, 2, 512], BF16, tag="maskB")
    nc.gpsimd.memset(maskB, 1.0)
    nc.gpsimd.affine_select(
        out=maskB, in_=maskB, compare_op=mybir.AluOpType.is_ge, fill=0.0,
        base=-256, pattern=[[-128, 2], [1, 512]], channel_multiplier=-1)
    eps_t = singles.tile([128, 1], F32, tag="eps_t")
    nc.vector.memset(eps_t, EPS)
    # DRAM bounce buffer for broadcasting softmax 1/denominator rows
    scr = nc.dram_tensor("scr", [16, 512], F32, kind="Internal").ap()

    # weights/LUT inputs first (small), then x in two T-halves so phase 1 can
    # start after the first half lands.
    wq_sb = singles.tile([128, KC, DQ], BF16, tag="wq_sb")
    nc.sync.dma_start(out=wq_sb, in_=wq.rearrange("(a p) n -> p a n", p=128))
    wkv_sb = singles.tile([128, KC, 128], BF16, tag="wkv_sb")
    nc.sync.dma_start(out=wkv_sb, in_=wkv.rearrange("(a p) n -> p a n", p=128))
    cos_sb = singles.tile([128, TC, 32], F32, tag="cos_sb")
    nc.sync.dma_start(out=cos_sb, in_=cos2.rearrange("(a p) d -> p a d", p=128))
    sin_sb = singles.tile([128, TC, 32], F32, tag="sin_sb")
    nc.sync.dma_start(out=sin_sb, in_=sin2.rearrange("(a p) d -> p a d", p=128))
    xsb = singles.tile([128, KC, T], BF16, tag="xsb")
    xT3 = xT.rearrange("(a p) t -> p a t", p=128)
    for q4 in range(4):
        nc.sync.dma_start(out=xsb[:, :, q4 * 512:(q4 + 1) * 512],
                          in_=xT3[:, :, q4 * 512:(q4 + 1) * 512])
    wo_sb = singles.tile([128, 2, C], BF16, tag="wo_sb")
    nc.sync.dma_start(out=wo_sb, in_=wo.rearrange("(a p) o -> p a o", p=128))

    # qkt[:, 0, :] = q heads 0,1 transposed; [:, 1, :] = q heads 2,3;
    # [:, 2, :] = [K^T; K^T] duplicated.
    qkt = singles.tile([128, 3, T], BF16, tag="qkt")
    v_sb = singles.tile([128, TC, 65], BF16, tag="v_sb")
    nc.vector.memset(v_sb[:, :, 64:65], 1.0)
    # yts[:, p, :] = normalized attention out for head pair p, [d, t] layout
    yts = singles.tile([128, 2, T], BF16, tag="yts")

    q2 = singles.tile([128, TC, DQ], F32, tag="q2")
    q2b = singles.tile([128, TC, DQ], BF16, tag="q2b")
    kn = singles.tile([128, TC, 64], F32, tag="kn")
    knb = singles.tile([128, TC, 128], BF16, tag="knb")
    mv = singles.tile([128, TC, HQ], F32, tag="mv")
    mvk = singles.tile([128, TC, 1], F32, tag="mvk")

    # ---- Phase 1: projections + rope + sumsq; rms+transpose per T-half ----
    with (
        tc.tile_pool(name="rtmp", bufs=2) as rtmp,
        tc.tile_pool(name="sqp", bufs=2) as sqp,
        tc.tile_pool(name="rmsp", bufs=2) as rmsp,
        tc.tile_pool(name="pp", bufs=2, space="PSUM") as pp,
        tc.tile_pool(name="tpp", bufs=3, space="PSUM") as tpp,
    ):
        def _rms_transpose(half):
            ts, te = 8 * half, 8 * half + 8
            sd = rmsp.tile([128, 8, HQ], F32, tag="sd")
            nc.scalar.activation(sd, mv[:, ts:te, :], ACT_SQRT,
                                 bias=eps_t, scale=1.0 / D)
            rsq = rmsp.tile([128, 8, HQ], F32, tag="rsq")
            nc.vector.reciprocal(rsq, sd)
            q4 = q2.rearrange("p t (h d) -> p t h d", d=D)
            q4b = q2b.rearrange("p t (h d) -> p t h d", d=D)
            nc.vector.tensor_mul(q4b[:, ts:ts + 5], q4[:, ts:ts + 5],
                                 _bcast_ap(rsq[:, 0:5], D, at=3))
            nc.gpsimd.tensor_mul(q4b[:, ts + 5:te], q4[:, ts + 5:te],
                                 _bcast_ap(rsq[:, 5:8], D, at=3))
            sdk = rmsp.tile([128, 8, 1], F32, tag="sdk")
            nc.scalar.activation(sdk, mvk[:, ts:te, :], ACT_SQRT,
                                 bias=eps_t, scale=1.0 / D)
            rsk = rmsp.tile([128, 8, 1], F32, tag="rsk")
            nc.vector.reciprocal(rsk, sdk)
            rkb = bass.AP(tensor=rsk.tensor, offset=rsk.offset,
                          ap=[rsk.ap[0], rsk.ap[1], [0, 64]])
            nc.vector.tensor_mul(knb[:, ts:te, 0:64], kn[:, ts:te, :], rkb)
            nc.gpsimd.tensor_copy(knb[:, ts:te, 64:128], knb[:, ts:te, 0:64])
            for t_ in range(ts, te):
                tps = tpp.tile([128, 3, 128], BF16, tag="tps")
                nc.tensor.transpose(tps[:, 0, :], q2b[:, t_, 0:128], ident)
                nc.tensor.transpose(tps[:, 1, :], q2b[:, t_, 128:256], ident)
                nc.tensor.transpose(tps[:, 2, :], knb[:, t_, :], ident)
                nc.scalar.copy(qkt[:, :, t_ * 128:(t_ + 1) * 128], tps)
            if half == 1:
                # preload exp table set while ACT is otherwise free
                dummy = rmsp.tile([1, 1], F32, tag="dummy")
                nc.scalar.activation(dummy, eps_t[0:1, 0:1], ACT_EXP)

        for grp in range(8):
            ta, tb = 2 * grp, 2 * grp + 2
            kvps = pp.tile([128, 2, 128], F32, tag="kvps")
            for tt in range(2):
                t_ = ta + tt
                for kc in range(KC):
                    nc.tensor.matmul(
                        kvps[:, tt, :], xsb[:, kc, t_ * 128:(t_ + 1) * 128],
                        wkv_sb[:, kc, :], start=(kc == 0), stop=(kc == KC - 1))
            qps = pp.tile([128, 2, DQ], F32, tag="qps")
            for tt in range(2):
                t_ = ta + tt
                for kc in range(KC):
                    nc.tensor.matmul(
                        qps[:, tt, :], xsb[:, kc, t_ * 128:(t_ + 1) * 128],
                        wq_sb[:, kc, :], start=(kc == 0), stop=(kc == KC - 1))
            # v out (ACT reads PSUM; gpsimd cannot)
            nc.scalar.copy(v_sb[:, ta:tb, 0:64], kvps[:, :, 64:128])
            # sumsq pre-rope (rope is norm-preserving)
            sq = sqp.tile([128, 2, DQ], F32, tag="sq")
            nc.scalar.activation(sq, qps, ACT_SQUARE)
            nc.vector.tensor_reduce(
                mv[:, ta:tb, :], sq.rearrange("p t (h d) -> p t h d", d=D),
                axis=mybir.AxisListType.X, op=mybir.AluOpType.add)
            sqk = sqp.tile([128, 2, 64], F32, tag="sqk")
            nc.scalar.activation(sqk, kvps[:, :, 0:64], ACT_SQUARE)
            nc.vector.tensor_reduce(mvk[:, ta:tb, :], sqk,
                                    axis=mybir.AxisListType.X,
                                    op=mybir.AluOpType.add)
            # rope q: [128, 2, 4, 32] ops
            q3 = qps.rearrange("p t (h d) -> p t h d", h=HQ)
            x1, x2 = q3[:, :, :, 0:32], q3[:, :, :, 32:64]
            cb = _bcast_ap(cos_sb[:, ta:tb, :], HQ, at=2)
            sb = _bcast_ap(sin_sb[:, ta:tb, :], HQ, at=2)
            o3 = q2[:, ta:tb, :].rearrange("p t (h d) -> p t h d", h=HQ)
            t1 = rtmp.tile([128, 2, HQ, 32], F32, tag="t1")
            t2 = rtmp.tile([128, 2, HQ, 32], F32, tag="t2")
            t3 = rtmp.tile([128, 2, HQ, 32], F32, tag="t3")
            t4 = rtmp.tile([128, 2, HQ, 32], F32, tag="t4")
            nc.vector.tensor_mul(t1, x1, cb)
            nc.vector.tensor_mul(t2, x2, sb)
            nc.vector.tensor_add(o3[:, :, :, 0:32], t1, t2)
            nc.vector.tensor_mul(t3, x1, sb)
            nc.vector.tensor_mul(t4, x2, cb)
            nc.vector.tensor_sub(o3[:, :, :, 32:64], t4, t3)
            # rope k: [128, 2, 32] (small, DVE)
            kx1, kx2 = kvps[:, :, 0:32], kvps[:, :, 32:64]
            kc_ = cos_sb[:, ta:tb, :]
            ks_ = sin_sb[:, ta:tb, :]
            u1 = rtmp.tile([128, 2, 32], F32, tag="u1")
            u2 = rtmp.tile([128, 2, 32], F32, tag="u2")
            u3 = rtmp.tile([128, 2, 32], F32, tag="u3")
            u4 = rtmp.tile([128, 2, 32], F32, tag="u4")
            nc.vector.tensor_mul(u1, kx1, kc_)
            nc.vector.tensor_mul(u2, kx2, ks_)
            nc.vector.tensor_add(kn[:, ta:tb, 0:32], u1, u2)
            nc.vector.tensor_mul(u3, kx1, ks_)
            nc.vector.tensor_mul(u4, kx2, kc_)
            nc.vector.tensor_sub(kn[:, ta:tb, 32:64], u4, u3)
            if grp == 3:
                _rms_transpose(0)
            elif grp == 7:
                _rms_transpose(1)

    # ---- Phase 2+3: attention + out projection per query block ----
    with (
        tc.tile_pool(name="ptp", bufs=3) as ptp,
        tc.tile_pool(name="smallp", bufs=4) as smallp,
        tc.tile_pool(name="bcsp", bufs=2) as bcsp,
        tc.tile_pool(name="osp", bufs=3) as osp,
        tc.tile_pool(name="s4p", bufs=2, space="PSUM") as s4p,
        tc.tile_pool(name="o65p", bufs=2, space="PSUM") as o65p,
        tc.tile_pool(name="tailp", bufs=2, space="PSUM") as tailp,
    ):
        # one-time init of the two s4 PSUM buffers: later diagonal blocks
        # leave stale (but bounded) values in column-restricted regions; the
        # very first uses would otherwise exp() raw PSUM garbage.
        for _ in range(2):
            sini = s4p.tile([128, 2, 512], F32, tag="s4")
            nc.vector.memset(sini, 0.0)
        def _outproj(jp):
            jqp = slice(jp * 512, (jp + 1) * 512)
            for m in range(8):
                ops_ = tailp.tile([128, 512], F32, tag="tail")
                for fc in range(2):
                    nc.tensor.matmul(
                        ops_, wo_sb[:, fc, m * 128:(m + 1) * 128],
                        yts[:, fc, jqp], start=(fc == 0), stop=(fc == 1))
                ot = osp.tile([128, 512], BF16, tag="ot")
                nc.vector.tensor_copy(ot, ops_)
                nc.sync.dma_start(
                    out=outT[m * 128:(m + 1) * 128, jqp], in_=ot)

        for j in range(NJ):
            jq = slice(j * 512, (j + 1) * 512)
            for h in range(HQ):
                pair, base = h // 2, (h % 2) * 64
                tp = (base, 0) if base else None
                o65 = o65p.tile([65, 512], F32, tag="o65")
                nblk = 2 * (j + 1)
                for g2 in range(nblk):
                    diag_half = g2 - 2 * j  # 0 -> chunks i=0,1; 1 -> i=2,3
                    s4 = s4p.tile([128, 2, 512], F32, tag="s4")
                    for i2 in range(2):
                        c = 2 * g2 + i2
                        i = c - 4 * j
                        lo = 128 * i if i > 0 else 0
                        nc.tensor.matmul(
                            s4[:, i2, lo:512],
                            qkt[base:base + 64, 2, c * 128:(c + 1) * 128],
                            qkt[base:base + 64, pair, j * 512 + lo:(j + 1) * 512],
                            start=True, stop=True, tile_position=tp)
                    pt = ptp.tile([128, 2, 512], BF16, tag="pt")
                    nc.scalar.activation(pt, s4, ACT_EXP, scale=0.125)
                    if diag_half >= 0:
                        msk = maskA if diag_half == 0 else maskB
                        nc.vector.tensor_mul(pt, pt, msk)
                    for i2 in range(2):
                        c = 2 * g2 + i2
                        i = c - 4 * j
                        lo = 128 * i if i > 0 else 0
                        nc.tensor.matmul(
                            o65[:, lo:512], v_sb[:, c, 0:65], pt[:, i2, lo:512],
                            start=(g2 == 0 and i2 == 0),
                            stop=(g2 == nblk - 1 and i2 == 1))
                # copy numerators out now so the o65 PSUM bank frees early;
                # the normalization mul happens later, off the critical path
                nc.vector.tensor_copy(
                    yts[base:base + 64, pair, jq], o65[0:64, :])
                rec = smallp.tile([1, 512], F32, tag="rec")
                if h == 3:
                    # 1/x = exp(-ln(x)); Ln/Exp share one ACT table set
                    lnt = smallp.tile([1, 512], F32, tag="lnt")
                    nc.scalar.activation(lnt, o65[64:65, :], ACT_LOG)
                    nc.scalar.activation(rec, lnt, ACT_EXP, scale=-1.0)
                else:
                    nc.vector.reciprocal(rec, o65[64:65, :])
                # broadcast 1/denominator to 64 partitions via a DRAM bounce
                idx = 4 * j + h
                nc.sync.dma_start(out=scr[idx:idx + 1, :], in_=rec)
                bcst = bcsp.tile([128, 512], F32, tag="bcs")
                bcs = bcst[base:base + 64, :]
                ssrc = scr[idx:idx + 1, :]
                nc.sync.dma_start(out=bcs, in_=bass.AP(
                    tensor=ssrc.tensor, offset=ssrc.offset,
                    ap=[[0, 64]] + list(ssrc.ap[1:])))
                nc.vector.tensor_mul(
                    yts[base:base + 64, pair, jq],
                    yts[base:base + 64, pair, jq], bcs)
                if h == 0 and j > 0:
                    _outproj(j - 1)  # previous block's out projection,
                    # deferred so its yts deps have a full iteration of slack
        _outproj(NJ - 1)


def _build_nc():
    nc = bass.Bass("TRN2", target_bir_lowering=False, debug=False, num_devices=8)
    ins = {
        "xT": nc.dram_tensor("xT", [1024, 2048], BF16, kind="ExternalInput").ap(),
        "wq": nc.dram_tensor("wq", [1024, 256], BF16, kind="ExternalInput").ap(),
        "wkv": nc.dram_tensor("wkv", [1024, 128], BF16, kind="ExternalInput").ap(),
        "wo": nc.dram_tensor("wo", [256, 1024], BF16, kind="ExternalInput").ap(),
        "cos2": nc.dram_tensor("cos2", [2048, 32], F32, kind="ExternalInput").ap(),
        "sin2": nc.dram_tensor("sin2", [2048, 32], F32, kind="ExternalInput").ap(),
    }
    outs = {"outT": nc.dram_tensor("outT", [1024, 2048], BF16,
                                   kind="ExternalOutput").ap()}
    with TileContext(nc) as tc:
        with ExitStack() as ctx:
            _build_attn(ctx, tc, outs, ins)
    _split_waits(nc, maxw=1)
    return nc


def _shard_inputs(inputs, b, g):
    x, cos, sin = inputs["x"], inputs["cos"], inputs["sin"]
    Wq, Wk, Wv, Wo = inputs["Wq"], inputs["Wk"], inputs["Wv"], inputs["Wo"]
    qs, ks = slice(g * 256, (g + 1) * 256), slice(g * 64, (g + 1) * 64)
    return {
        "xT": np.ascontiguousarray(np.asarray(x[b]).T.astype(NPBF16)),
        "wq": np.ascontiguousarray(np.asarray(Wq[qs]).T.astype(NPBF16)),
        "wkv": np.ascontiguousarray(np.concatenate(
            [np.asarray(Wk[ks]).T, np.asarray(Wv[ks]).T], axis=1).astype(NPBF16)),
        "wo": np.ascontiguousarray(np.asarray(Wo[:, qs]).T.astype(NPBF16)),
        "cos2": np.ascontiguousarray(np.asarray(cos[0, :, 0, :]), dtype=np.float32),
        "sin2": np.ascontiguousarray(np.asarray(sin[0, :, 0, :]), dtype=np.float32),
    }


_STATE = None


def _get_state():
    global _STATE
    if _STATE is not None:
        return _STATE
    import jax
    from jax.sharding import Mesh, PartitionSpec, NamedSharding
    from jax.experimental.shard_map import shard_map
    from concourse.bass2jax import (
        _bass_exec_p, install_neuronx_cc_hook, partition_id_tensor)

    install_neuronx_cc_hook()
    nc = _build_nc()
    pname = nc.partition_id_tensor.name if nc.partition_id_tensor else None

    in_names, out_names, out_avals, zero_outs = [], [], [], []
    for alloc in nc.m.functions[0].allocations:
        if not isinstance(alloc, mybir.MemoryLocationSet):
            continue
        name = alloc.memorylocations[0].name
        if alloc.kind == "ExternalInput":
            if name != pname:
                in_names.append(name)
        elif alloc.kind == "ExternalOutput":
            out_names.append(name)
            shape = tuple(alloc.tensor_shape)
            dtype = mybir.dt.np(alloc.dtype)
            out_avals.append(jax.core.ShapedArray(shape, dtype))
            zero_outs.append(np.zeros(shape, dtype))
    n_params = len(in_names)
    all_names = in_names + out_names
    if pname is not None:
        all_names = all_names + [pname]

    def _body(*args):
        operands = list(args)
        if pname is not None:
            operands.append(partition_id_tensor())
        outs = _bass_exec_p.bind(
            *operands, out_avals=tuple(out_avals), in_names=tuple(all_names),
            out_names=tuple(out_names), lowering_input_output_aliases=(),
            sim_require_finite=True, sim_require_nnan=True, nc=nc)
        return tuple(outs)

    devices = jax.devices()[:8]
    mesh = Mesh(np.asarray(devices), ("core",))
    specs = (PartitionSpec("core"),) * (n_params + 1)
    sharded = jax.jit(shard_map(_body, mesh=mesh, in_specs=specs,
                                out_specs=(PartitionSpec("core"),),
                                check_rep=False))
    sharding = NamedSharding(mesh, PartitionSpec("core"))
    zeros = jax.device_put(
        np.zeros((8 * 1024, 2048), NPBF16), sharding)
    _STATE = dict(sharded=sharded, sharding=sharding, in_names=in_names,
                  zeros=zeros, jax=jax)
    return _STATE


def _run_device(in_maps):
    st = _get_state()
    jax = st["jax"]
    concat_in = [np.concatenate([m[n] for m in in_maps], axis=0)
                 for n in st["in_names"]]
    dev_in = [jax.device_put(a, st["sharding"]) for a in concat_in]
    out = st["sharded"](*dev_in, st["zeros"])[0]
    return np.asarray(out).reshape(8, 1024, 2048)


def kernel(**inputs) -> np.ndarray:
    inputs = {k: np.asarray(v) for k, v in inputs.items()}
    in_maps = [_shard_inputs(inputs, b, g) for b in range(2) for g in range(4)]
    arr = _run_device(in_maps)
    out = np.zeros((2, 2048, 1024), np.float32)
    for c in range(8):
        out[c // 4] += arr[c].T.astype(np.float32)
    return out


# revision 36
# speedup vs baseline: 1.0322x; 1.0071x over previous
"""Trainium2 Bass kernel: causal GQA self-attention (B=2, T=2048, C=1024,
16 q-heads / 4 kv-heads, rotary + q/k RMS-norm), sharded over 8 NeuronCores
as (batch x kv-group). Self-contained: kernel(**inputs) -> np.ndarray.
"""
import sys
from contextlib import ExitStack

for p in ("/opt/trn_rl_repo", "/root/.axon_site/_ro/trn_rl_repo"):
    if p not in sys.path:
        sys.path.insert(0, p)

import numpy as np
import ml_dtypes

import concourse.bass as bass
import concourse.mybir as mybir
from concourse.tile import TileContext
from concourse.masks import make_identity

F32 = mybir.dt.float32
BF16 = mybir.dt.bfloat16
NPBF16 = ml_dtypes.bfloat16

T, C, HQ, D = 2048, 1024, 4, 64
DQ = HQ * D
TC = T // 128
KC = C // 128
NJ = T // 512
EPS = 1.1920929e-7
ACT_SQUARE = mybir.ActivationFunctionType.Square
ACT_SQRT = mybir.ActivationFunctionType.Sqrt
ACT_EXP = mybir.ActivationFunctionType.Exp
ACT_LOG = mybir.ActivationFunctionType.Ln


def _bcast_ap(sl, n, at=1):
    ap = list(sl.ap)
    ap.insert(at, [0, n])
    return bass.AP(tensor=sl.tensor, offset=sl.offset, ap=ap)


def _split_waits(nc, maxw=1):
    """Walrus in this toolchain allows 1 sem-wait per instruction; split extras
    onto preceding same-engine NoOps."""
    cnt = 0
    for f in nc.m.functions:
        for b in f.blocks:
            il = list(b.instructions)
            out = []
            changed = False
            for inst in il:
                si = inst.sync_info
                waits = list(si.on_wait) if si and si.on_wait else []
                if len(waits) > maxw:
                    chunks = [waits[i:i + maxw] for i in range(0, len(waits), maxw)]
                    for ch in chunks[:-1]:
                        cnt += 1
                        nop = mybir.InstNoOp(name=f"I-waitfix-{cnt}")
                        nop.engine = inst.engine
                        nop.sync_info = mybir.SyncInfo(on_wait=ch, on_update=[])
                        out.append(nop)
                    si.on_wait = chunks[-1]
                    inst.sync_info = si
                    changed = True
                out.append(inst)
            if changed:
                b.instructions = out
    return cnt


def _build_attn(ctx, tc, outs, ins):
    nc = tc.nc
    xT, wq, wkv, wo, cos2, sin2 = (
        ins["xT"], ins["wq"], ins["wkv"], ins["wo"], ins["cos2"], ins["sin2"])
    outT = outs["outT"]

    singles = ctx.enter_context(tc.tile_pool(name="singles", bufs=1))

    ident = singles.tile([128, 128], BF16, tag="ident")
    make_identity(nc, ident)
    # 0/1 bf16 causal masks for the two diagonal half-blocks.
    # maskA: key chunks (4j+0, 4j+1); maskB: key chunks (4j+2, 4j+3).
    # keep (1.0) where q - 128*(slot + 2*half) - p >= 0 else 0.0
    maskA = singles.tile([128, 2, 512], BF16, tag="maskA")
    nc.gpsimd.memset(maskA, 1.0)
    nc.gpsimd.affine_select(
        out=maskA, in_=maskA, compare_op=mybir.AluOpType.is_ge, fill=0.0,
        base=0, pattern=[[-128, 2], [1, 512]], channel_multiplier=-1)
    maskB = singles.tile([128, 2, 512], BF16, tag="maskB")
    nc.gpsimd.memset(maskB, 1.0)
    nc.gpsimd.affine_select(
        out=maskB, in_=maskB, compare_op=mybir.AluOpType.is_ge, fill=0.0,
        base=-256, pattern=[[-128, 2], [1, 512]], channel_multiplier=-1)
    eps_t = singles.tile([128, 1], F32, tag="eps_t")
    nc.vector.memset(eps_t, EPS)
    # DRAM bounce buffer for broadcasting softmax 1/denominator rows
    scr = nc.dram_tensor("scr", [16, 512], F32, kind="Internal").ap()

    # weights/LUT inputs first (small), then x in two T-halves so phase 1 can
    # start after the first half lands.
    wq_sb = singles.tile([128, KC, DQ], BF16, tag="wq_sb")
    nc.sync.dma_start(out=wq_sb, in_=wq.rearrange("(a p) n -> p a n", p=128))
    wkv_sb = singles.tile([128, KC, 128], BF16, tag="wkv_sb")
    nc.sync.dma_start(out=wkv_sb, in_=wkv.rearrange("(a p) n -> p a n", p=128))
    cos_sb = singles.tile([128, TC, 32], F32, tag="cos_sb")
    nc.sync.dma_start(out=cos_sb, in_=cos2.rearrange("(a p) d -> p a d", p=128))
    sin_sb = singles.tile([128, TC, 32], F32, tag="sin_sb")
    nc.sync.dma_start(out=sin_sb, in_=sin2.rearrange("(a p) d -> p a d", p=128))
    xsb = singles.tile([128, KC, T], BF16, tag="xsb")
    xT3 = xT.rearrange("(a p) t -> p a t", p=128)
    xcuts = [0, 256, 512, 1024, 1536, 2048]
    for xa, xb in zip(xcuts[:-1], xcuts[1:]):
        nc.sync.dma_start(out=xsb[:, :, xa:xb], in_=xT3[:, :, xa:xb])
    wo_sb = singles.tile([128, 2, C], BF16, tag="wo_sb")
    nc.sync.dma_start(out=wo_sb, in_=wo.rearrange("(a p) o -> p a o", p=128))

    # qkt[:, 0, :] = q heads 0,1 transposed; [:, 1, :] = q heads 2,3;
    # [:, 2, :] = [K^T; K^T] duplicated.
    qkt = singles.tile([128, 3, T], BF16, tag="qkt")
    v_sb = singles.tile([128, TC, 65], BF16, tag="v_sb")
    nc.vector.memset(v_sb[:, :, 64:65], 1.0)
    # yts[:, p, :] = normalized attention out for head pair p, [d, t] layout
    yts = singles.tile([128, 2, T], BF16, tag="yts")

    q2 = singles.tile([128, TC, DQ], F32, tag="q2")
    q2b = singles.tile([128, TC, DQ], BF16, tag="q2b")
    kn = singles.tile([128, TC, 64], F32, tag="kn")
    knb = singles.tile([128, TC, 128], BF16, tag="knb")
    mv = singles.tile([128, TC, HQ], F32, tag="mv")
    mvk = singles.tile([128, TC, 1], F32, tag="mvk")

    # ---- Phase 1: projections + rope + sumsq; rms+transpose per T-half ----
    with (
        tc.tile_pool(name="rtmp", bufs=2) as rtmp,
        tc.tile_pool(name="sqp", bufs=2) as sqp,
        tc.tile_pool(name="rmsp", bufs=2) as rmsp,
        tc.tile_pool(name="pp", bufs=2, space="PSUM") as pp,
        tc.tile_pool(name="tpp", bufs=3, space="PSUM") as tpp,
    ):
        def _rms_transpose(half):
            ts, te = 8 * half, 8 * half + 8
            sd = rmsp.tile([128, 8, HQ], F32, tag="sd")
            nc.scalar.activation(sd, mv[:, ts:te, :], ACT_SQRT,
                                 bias=eps_t, scale=1.0 / D)
            rsq = rmsp.tile([128, 8, HQ], F32, tag="rsq")
            nc.vector.reciprocal(rsq, sd)
            q4 = q2.rearrange("p t (h d) -> p t h d", d=D)
            q4b = q2b.rearrange("p t (h d) -> p t h d", d=D)
            nc.vector.tensor_mul(q4b[:, ts:ts + 5], q4[:, ts:ts + 5],
                                 _bcast_ap(rsq[:, 0:5], D, at=3))
            nc.gpsimd.tensor_mul(q4b[:, ts + 5:te], q4[:, ts + 5:te],
                                 _bcast_ap(rsq[:, 5:8], D, at=3))
            sdk = rmsp.tile([128, 8, 1], F32, tag="sdk")
            nc.scalar.activation(sdk, mvk[:, ts:te, :], ACT_SQRT,
                                 bias=eps_t, scale=1.0 / D)
            rsk = rmsp.tile([128, 8, 1], F32, tag="rsk")
            nc.vector.reciprocal(rsk, sdk)
            rkb = bass.AP(tensor=rsk.tensor, offset=rsk.offset,
                          ap=[rsk.ap[0], rsk.ap[1], [0, 64]])
            nc.vector.tensor_mul(knb[:, ts:te, 0:64], kn[:, ts:te, :], rkb)
            nc.gpsimd.tensor_copy(knb[:, ts:te, 64:128], knb[:, ts:te, 0:64])
            for t_ in range(ts, te):
                tps = tpp.tile([128, 3, 128], BF16, tag="tps")
                nc.tensor.transpose(tps[:, 0, :], q2b[:, t_, 0:128], ident)
                nc.tensor.transpose(tps[:, 1, :], q2b[:, t_, 128:256], ident)
                nc.tensor.transpose(tps[:, 2, :], knb[:, t_, :], ident)
                nc.scalar.copy(qkt[:, :, t_ * 128:(t_ + 1) * 128], tps)
            if half == 1:
                # preload exp table set while ACT is otherwise free
                dummy = rmsp.tile([1, 1], F32, tag="dummy")
                nc.scalar.activation(dummy, eps_t[0:1, 0:1], ACT_EXP)

        for grp in range(8):
            ta, tb = 2 * grp, 2 * grp + 2
            kvps = pp.tile([128, 2, 128], F32, tag="kvps")
            for tt in range(2):
                t_ = ta + tt
                for kc in range(KC):
                    nc.tensor.matmul(
                        kvps[:, tt, :], xsb[:, kc, t_ * 128:(t_ + 1) * 128],
                        wkv_sb[:, kc, :], start=(kc == 0), stop=(kc == KC - 1))
            qps = pp.tile([128, 2, DQ], F32, tag="qps")
            for tt in range(2):
                t_ = ta + tt
                for kc in range(KC):
                    nc.tensor.matmul(
                        qps[:, tt, :], xsb[:, kc, t_ * 128:(t_ + 1) * 128],
                        wq_sb[:, kc, :], start=(kc == 0), stop=(kc == KC - 1))
            # v out (ACT reads PSUM; gpsimd cannot)
            nc.scalar.copy(v_sb[:, ta:tb, 0:64], kvps[:, :, 64:128])
            # stage q in SBUF so gpsimd can take part of the rope work
            q_s = sqp.tile([128, 2, DQ], F32, tag="qs")
            nc.scalar.copy(q_s, qps)
            # sumsq pre-rope (rope is norm-preserving)
            sq = sqp.tile([128, 2, DQ], F32, tag="sq")
            nc.scalar.activation(sq, qps, ACT_SQUARE)
            nc.vector.tensor_reduce(
                mv[:, ta:tb, :], sq.rearrange("p t (h d) -> p t h d", d=D),
                axis=mybir.AxisListType.X, op=mybir.AluOpType.add)
            sqk = sqp.tile([128, 2, 64], F32, tag="sqk")
            nc.scalar.activation(sqk, kvps[:, :, 0:64], ACT_SQUARE)
            nc.vector.tensor_reduce(mvk[:, ta:tb, :], sqk,
                                    axis=mybir.AxisListType.X,
                                    op=mybir.AluOpType.add)
            # rope q: [128, 2, 4, 32] ops, two muls offloaded to gpsimd
            q3 = q_s.rearrange("p t (h d) -> p t h d", h=HQ)
            x1, x2 = q3[:, :, :, 0:32], q3[:, :, :, 32:64]
            cb = _bcast_ap(cos_sb[:, ta:tb, :], HQ, at=2)
            sb = _bcast_ap(sin_sb[:, ta:tb, :], HQ, at=2)
            o3 = q2[:, ta:tb, :].rearrange("p t (h d) -> p t h d", h=HQ)
            t1 = rtmp.tile([128, 2, HQ, 32], F32, tag="t1")
            t2 = rtmp.tile([128, 2, HQ, 32], F32, tag="t2")
            t3 = rtmp.tile([128, 2, HQ, 32], F32, tag="t3")
            t4 = rtmp.tile([128, 2, HQ, 32], F32, tag="t4")
            nc.vector.tensor_mul(t1, x1, cb)
            nc.gpsimd.tensor_mul(t2, x2, sb)
            nc.vector.tensor_add(o3[:, :, :, 0:32], t1, t2)
            nc.gpsimd.tensor_mul(t3, x1, sb)
            nc.vector.tensor_mul(t4, x2, cb)
            nc.vector.tensor_sub(o3[:, :, :, 32:64], t4, t3)
            # rope k: [128, 2, 32] (small, DVE)
            kx1, kx2 = kvps[:, :, 0:32], kvps[:, :, 32:64]
            kc_ = cos_sb[:, ta:tb, :]
            ks_ = sin_sb[:, ta:tb, :]
            u1 = rtmp.tile([128, 2, 32], F32, tag="u1")
            u2 = rtmp.tile([128, 2, 32], F32, tag="u2")
            u3 = rtmp.tile([128, 2, 32], F32, tag="u3")
            u4 = rtmp.tile([128, 2, 32], F32, tag="u4")
            nc.vector.tensor_mul(u1, kx1, kc_)
            nc.vector.tensor_mul(u2, kx2, ks_)
            nc.vector.tensor_add(kn[:, ta:tb, 0:32], u1, u2)
            nc.vector.tensor_mul(u3, kx1, ks_)
            nc.vector.tensor_mul(u4, kx2, kc_)
            nc.vector.tensor_sub(kn[:, ta:tb, 32:64], u4, u3)
            if grp == 3:
                _rms_transpose(0)
            elif grp == 7:
                _rms_transpose(1)

    # ---- Phase 2+3: attention + out projection per query block ----
    with (
        tc.tile_pool(name="ptp", bufs=3) as ptp,
        tc.tile_pool(name="smallp", bufs=4) as smallp,
        tc.tile_pool(name="bcsp", bufs=2) as bcsp,
        tc.tile_pool(name="osp", bufs=3) as osp,
        tc.tile_pool(name="s4p", bufs=2, space="PSUM") as s4p,
        tc.tile_pool(name="o65p", bufs=2, space="PSUM") as o65p,
        tc.tile_pool(name="tailp", bufs=2, space="PSUM") as tailp,
    ):
        # one-time init of the two s4 PSUM buffers: later diagonal blocks
        # leave stale (but bounded) values in column-restricted regions; the
        # very first uses would otherwise exp() raw PSUM garbage.
        for _ in range(2):
            sini = s4p.tile([128, 2, 512], F32, tag="s4")
            nc.vector.memset(sini, 0.0)
        def _outproj(jp):
            jqp = slice(jp * 512, (jp + 1) * 512)
            for m in range(8):
                ops_ = tailp.tile([128, 512], F32, tag="tail")
                for fc in range(2):
                    nc.tensor.matmul(
                        ops_, wo_sb[:, fc, m * 128:(m + 1) * 128],
                        yts[:, fc, jqp], start=(fc == 0), stop=(fc == 1))
                ot = osp.tile([128, 512], BF16, tag="ot")
                nc.vector.tensor_copy(ot, ops_)
                nc.sync.dma_start(
                    out=outT[m * 128:(m + 1) * 128, jqp], in_=ot)

        for j in range(NJ):
            jq = slice(j * 512, (j + 1) * 512)
            for h in range(HQ):
                pair, base = h // 2, (h % 2) * 64
                tp = (base, 0) if base else None
                o65 = o65p.tile([65, 512], F32, tag="o65")
                nblk = 2 * (j + 1)
                for g2 in range(nblk):
                    diag_half = g2 - 2 * j  # 0 -> chunks i=0,1; 1 -> i=2,3
                    s4 = s4p.tile([128, 2, 512], F32, tag="s4")
                    for i2 in range(2):
                        c = 2 * g2 + i2
                        i = c - 4 * j
                        lo = 128 * i if i > 0 else 0
                        nc.tensor.matmul(
                            s4[:, i2, lo:512],
                            qkt[base:base + 64, 2, c * 128:(c + 1) * 128],
                            qkt[base:base + 64, pair, j * 512 + lo:(j + 1) * 512],
                            start=True, stop=True, tile_position=tp)
                    pt = ptp.tile([128, 2, 512], BF16, tag="pt")
                    nc.scalar.activation(pt, s4, ACT_EXP, scale=0.125)
                    if diag_half >= 0:
                        msk = maskA if diag_half == 0 else maskB
                        nc.vector.tensor_mul(pt, pt, msk)
                    for i2 in range(2):
                        c = 2 * g2 + i2
                        i = c - 4 * j
                        lo = 128 * i if i > 0 else 0
                        nc.tensor.matmul(
                            o65[:, lo:512], v_sb[:, c, 0:65], pt[:, i2, lo:512],
                            start=(g2 == 0 and i2 == 0),
                            stop=(g2 == nblk - 1 and i2 == 1))
                # copy numerators out now so the o65 PSUM bank frees early;
                # the normalization mul happens later, off the critical path
                nc.vector.tensor_copy(
                    yts[base:base + 64, pair, jq], o65[0:64, :])
                rec = smallp.tile([1, 512], F32, tag="rec")
                if h == 3:
                    # 1/x = exp(-ln(x)); Ln/Exp share one ACT table set
                    lnt = smallp.tile([1, 512], F32, tag="lnt")
                    nc.scalar.activation(lnt, o65[64:65, :], ACT_LOG)
                    nc.scalar.activation(rec, lnt, ACT_EXP, scale=-1.0)
                else:
                    nc.vector.reciprocal(rec, o65[64:65, :])
                # broadcast 1/denominator to 64 partitions via a DRAM bounce
                idx = 4 * j + h
                nc.sync.dma_start(out=scr[idx:idx + 1, :], in_=rec)
                bcst = bcsp.tile([128, 512], F32, tag="bcs")
                bcs = bcst[base:base + 64, :]
                ssrc = scr[idx:idx + 1, :]
                nc.sync.dma_start(out=bcs, in_=bass.AP(
                    tensor=ssrc.tensor, offset=ssrc.offset,
                    ap=[[0, 64]] + list(ssrc.ap[1:])))
                nc.vector.tensor_mul(
                    yts[base:base + 64, pair, jq],
                    yts[base:base + 64, pair, jq], bcs)
                if h == 0 and j > 0:
                    _outproj(j - 1)  # previous block's out projection,
                    # deferred so its yts deps have a full iteration of slack
        _outproj(NJ - 1)


def _build_nc():
    nc = bass.Bass("TRN2", target_bir_lowering=False, debug=False, num_devices=8)
    ins = {
        "xT": nc.dram_tensor("xT", [1024, 2048], BF16, kind="ExternalInput").ap(),
        "wq": nc.dram_tensor("wq", [1024, 256], BF16, kind="ExternalInput").ap(),
        "wkv": nc.dram_tensor("wkv", [1024, 128], BF16, kind="ExternalInput").ap(),
        "wo": nc.dram_tensor("wo", [256, 1024], BF16, kind="ExternalInput").ap(),
        "cos2": nc.dram_tensor("cos2", [2048, 32], F32, kind="ExternalInput").ap(),
        "sin2": nc.dram_tensor("sin2", [2048, 32], F32, kind="ExternalInput").ap(),
    }
    outs = {"outT": nc.dram_tensor("outT", [1024, 2048], BF16,
                                   kind="ExternalOutput").ap()}
    with TileContext(nc) as tc:
        with ExitStack() as ctx:
            _build_attn(ctx, tc, outs, ins)
    _split_waits(nc, maxw=1)
    return nc


def _shard_inputs(inputs, b, g):
    x, cos, sin = inputs["x"], inputs["cos"], inputs["sin"]
    Wq, Wk, Wv, Wo = inputs["Wq"], inputs["Wk"], inputs["Wv"], inputs["Wo"]
    qs, ks = slice(g * 256, (g + 1) * 256), slice(g * 64, (g + 1) * 64)
    return {
        "xT": np.ascontiguousarray(np.asarray(x[b]).T.astype(NPBF16)),
        "wq": np.ascontiguousarray(np.asarray(Wq[qs]).T.astype(NPBF16)),
        "wkv": np.ascontiguousarray(np.concatenate(
            [np.asarray(Wk[ks]).T, np.asarray(Wv[ks]).T], axis=1).astype(NPBF16)),
        "wo": np.ascontiguousarray(np.asarray(Wo[:, qs]).T.astype(NPBF16)),
        "cos2": np.ascontiguousarray(np.asarray(cos[0, :, 0, :]), dtype=np.float32),
        "sin2": np.ascontiguousarray(np.asarray(sin[0, :, 0, :]), dtype=np.float32),
    }


_STATE = None


def _get_state():
    global _STATE
    if _STATE is not None:
        return _STATE
    import jax
    from jax.sharding import Mesh, PartitionSpec, NamedSharding
    from jax.experimental.shard_map import shard_map
    from concourse.bass2jax import (
        _bass_exec_p, install_neuronx_cc_hook, partition_id_tensor)

    install_neuronx_cc_hook()
    nc = _build_nc()
    pname = nc.partition_id_tensor.name if nc.partition_id_tensor else None

    in_names, out_names, out_avals, zero_outs = [], [], [], []
    for alloc in nc.m.functions[0].allocations:
        if not isinstance(alloc, mybir.MemoryLocationSet):
            continue
        name = alloc.memorylocations[0].name
        if alloc.kind == "ExternalInput":
            if name != pname:
                in_names.append(name)
        elif alloc.kind == "ExternalOutput":
            out_names.append(name)
            shape = tuple(alloc.tensor_shape)
            dtype = mybir.dt.np(alloc.dtype)
            out_avals.append(jax.core.ShapedArray(shape, dtype))
            zero_outs.append(np.zeros(shape, dtype))
    n_params = len(in_names)
    all_names = in_names + out_names
    if pname is not None:
        all_names = all_names + [pname]

    def _body(*args):
        operands = list(args)
        if pname is not None:
            operands.append(partition_id_tensor())
        outs = _bass_exec_p.bind(
            *operands, out_avals=tuple(out_avals), in_names=tuple(all_names),
            out_names=tuple(out_names), lowering_input_output_aliases=(),
            sim_require_finite=True, sim_require_nnan=True, nc=nc)
        return tuple(outs)

    devices = jax.devices()[:8]
    mesh = Mesh(np.asarray(devices), ("core",))
    specs = (PartitionSpec("core"),) * (n_params + 1)
    sharded = jax.jit(shard_map(_body, mesh=mesh, in_specs=specs,
                                out_specs=(PartitionSpec("core"),),
                                check_rep=False))
    sharding = NamedSharding(mesh, PartitionSpec("core"))
    zeros = jax.device_put(
        np.zeros((8 * 1024, 2048), NPBF16), sharding)
    _STATE = dict(sharded=sharded, sharding=sharding, in_names=in_names,
                  zeros=zeros, jax=jax)
    return _STATE


def _run_device(in_maps):
    st = _get_state()
    jax = st["jax"]
    concat_in = [np.concatenate([m[n] for m in in_maps], axis=0)
                 for n in st["in_names"]]
    dev_in = [jax.device_put(a, st["sharding"]) for a in concat_in]
    out = st["sharded"](*dev_in, st["zeros"])[0]
    return np.asarray(out).reshape(8, 1024, 2048)


def kernel(**inputs) -> np.ndarray:
    inputs = {k: np.asarray(v) for k, v in inputs.items()}
    in_maps = [_shard_inputs(inputs, b, g) for b in range(2) for g in range(4)]
    arr = _run_device(in_maps)
    out = np.zeros((2, 2048, 1024), np.float32)
    for c in range(8):
        out[c // 4] += arr[c].T.astype(np.float32)
    return out
